# revision 34
# baseline (speedup 1.0000x reference)
"""A3C loss kernel for Trainium2 (8 NeuronCores, data-parallel over batch).

The reference is a reverse scan over T=128 timesteps per trajectory:
    R_t   = sum_{s>=t} g^(s-t) r_s + g^(T-t) R0
    gae_t = sum_{s>=t} g^(s-t) delta_s,  delta_s = r_s + g*v_{s+1} - v_s (v_T = R0)
    critic = 0.5 * sum_t (R_t - v_t)^2
    actor  = -sum_t lp_t * gae_t - beta * sum_{t,a} ent
Both suffix scans are matmuls with a [T,T] discount matrix (gae telescopes to
adv = R - v), so the loss is pure streaming: 72 MB/core of HBM reads feeding a
handful of reductions and two small matmuls per 128-row block.

This version is tuned for DMA throughput (the kernel is hard memory-bound;
measured ~424 GB/s/core sustained): blocks are loaded in groups of 8 so each
dma_start moves 32 KB-contiguous chunks per partition, every stream load is
issued from the SP sequencer (which executes nothing but dma_starts, so DMA
issue never serializes behind compute), rewards stay resident in SBUF for the
whole kernel, and the final group is loaded per-block so the dependency chain
after the last HBM byte is short.
"""

import numpy as np
from contextlib import ExitStack

import concourse.bacc as bacc
import concourse.bass as bass
import concourse.tile as tile
from concourse import mybir
from concourse.bass_utils import run_bass_kernel_spmd

GAMMA = 0.99
BETA = 0.01
B, T, A = 65536, 128, 8
N_CORES = 8
BC = B // N_CORES

F32 = mybir.dt.float32
ALU = mybir.AluOpType
ACTF = mybir.ActivationFunctionType
AX = mybir.AxisListType

G = 8            # blocks per DMA group
KB = BC // 128   # 64 blocks of 128 rows
NG = KB // G     # 8 groups


def _discount_matrix() -> np.ndarray:
    # L[s, t] = gamma^(s-t) for s >= t else 0
    s = np.arange(T, dtype=np.float64)[:, None]
    t = np.arange(T, dtype=np.float64)[None, :]
    m = np.where(s >= t, GAMMA ** np.maximum(s - t, 0.0), 0.0)
    return m.astype(np.float32)


def build_nc(bc: int = BC):
    assert bc == 128 * KB

    nc = bacc.Bacc("TRN2", target_bir_lowering=False, debug=False)

    v_d = nc.dram_tensor("values", [bc, T], F32, kind="ExternalInput")
    lv_d = nc.dram_tensor("last_value", [bc], F32, kind="ExternalInput")
    r_d = nc.dram_tensor("rewards", [bc, T], F32, kind="ExternalInput")
    lp_d = nc.dram_tensor("log_probs", [bc, T, A], F32, kind="ExternalInput")
    en_d = nc.dram_tensor("entropies", [bc, T, A], F32, kind="ExternalInput")
    tm_d = nc.dram_tensor("terminal_mask", [bc], mybir.dt.uint8, kind="ExternalInput")
    out_d = nc.dram_tensor("out", [bc, 2], F32, kind="ExternalOutput")

    lgam_d = nc.inline_tensor(_discount_matrix(), "lgam")
    iden_d = nc.inline_tensor(np.eye(128, dtype=np.float32), "iden")

    # row b = p*KB + g*G + k  (partition p, group g, block j = g*G+k)
    lp_view = lp_d.rearrange("(p g k) t a -> g p (k t a)", g=NG, k=G)
    en_view = en_d.rearrange("(p g k) t a -> g p (k t a)", g=NG, k=G)
    v_view = v_d.rearrange("(p g k) t -> g p (k t)", g=NG, k=G)
    r_view = r_d.rearrange("(p j) t -> p (j t)", j=KB)
    lv_view = lv_d.rearrange("(p j) -> p j", j=KB)
    tm_view = tm_d.rearrange("(p j) -> p j", j=KB)
    out_view = out_d.rearrange("(p j) c -> p (j c)", j=KB)

    with tile.TileContext(nc) as tc, ExitStack() as ctx:
        singles = ctx.enter_context(tc.tile_pool(name="singles", bufs=1))
        streams = ctx.enter_context(tc.tile_pool(name="streams", bufs=2))
        work = ctx.enter_context(tc.tile_pool(name="work", bufs=2))
        psum = ctx.enter_context(tc.tile_pool(name="psum", bufs=2, space="PSUM"))

        # write-only scratch blocks: engines visit them serially, so one
        # block-sized dummy per op kind is enough (saves SBUF for the 4 MB
        # double-buffered stream tiles)
        enscr = singles.tile([128, T * A], F32)
        sqscr = singles.tile([128, T], F32)
        prod = singles.tile([128, T], F32)

        # rewards stay resident: one 4 MB load (32 KB/partition), issued first
        # ON THE SP RING so it drains at full rate before the streams (on the
        # ACT ring it would share engine bandwidth with the SP stream and
        # finish ~10us later, stalling the group-0 chain). ALL stream loads go
        # through the SP sequencer: it executes nothing but dma_starts, so it
        # runs ahead of compute and keeps the SDMA engines fed.
        r_s = singles.tile([128, KB * T], F32)
        nc.sync.dma_start(out=r_s, in_=r_view)

        # small one-time loads go through SWDGE (gpsimd) so they stay out of
        # both HWDGE FIFOs that carry the big streams
        lgam_s = singles.tile([128, 128], F32)
        nc.gpsimd.dma_start(out=lgam_s, in_=lgam_d[:])
        iden_s = singles.tile([128, 128], F32)
        nc.gpsimd.dma_start(out=iden_s, in_=iden_d[:])
        lv_s = singles.tile([128, KB], F32)
        nc.gpsimd.dma_start(out=lv_s, in_=lv_view)
        tm_s = singles.tile([128, KB], mybir.dt.uint8)
        nc.gpsimd.dma_start(out=tm_s, in_=tm_view)

        # gr0 = gamma * last_value * (1 - mask)
        tmf = singles.tile([128, KB], F32)
        nc.gpsimd.tensor_copy(out=tmf, in_=tm_s)
        lvm = singles.tile([128, KB], F32)
        nc.gpsimd.tensor_mul(lvm, lv_s, tmf)
        gr0 = singles.tile([128, KB], F32)
        nc.gpsimd.tensor_sub(gr0, lv_s, lvm)
        nc.gpsimd.tensor_scalar_mul(gr0, gr0, GAMMA)

        stage = singles.tile([128, 2 * KB], F32)

        for g in range(NG):
            tail = g == NG - 1
            gc = slice(g * G * T, (g + 1) * G * T)  # this group's v/r columns

            # loads (all on SP). The last group loads per-block so the
            # post-stream dependency chain is short. v gets an extra buffer:
            # its slot is released by the sub at the END of the per-group
            # dependency chain, so bufs=2 couples the prefetch to the chain.
            v_t = streams.tile([128, G * T], F32, tag="v", bufs=3)
            nc.sync.dma_start(out=v_t, in_=v_view[g])
            lp_t = streams.tile([128, G * T, A], F32, tag="lp")
            en_t = streams.tile([128, G * T * A], F32, tag="en")
            if tail:
                for k in range(G):
                    bs = slice(k * T * A, (k + 1) * T * A)
                    nc.sync.dma_start(out=en_t[:, bs], in_=en_view[g][:, bs])
                    if k < G - 1:
                        nc.sync.dma_start(out=lp_t[:, k * T : (k + 1) * T, :],
                                          in_=lp_view[g][:, bs])
                    else:
                        # halve the very last load so the post-stream reduce
                        # chain starts on the first half early
                        for h in range(2):
                            hh = slice(k * T * A + h * T * A // 2,
                                       k * T * A + (h + 1) * T * A // 2)
                            nc.sync.dma_start(
                                out=lp_t[:, k * T + h * T // 2 : k * T + (h + 1) * T // 2, :],
                                in_=lp_view[g][:, hh])
            else:
                nc.sync.dma_start(out=en_t, in_=en_view[g])
                nc.sync.dma_start(out=lp_t, in_=lp_view[g])

            # fold gamma*R0 into the last timestep of r, then time-major
            # transpose of all blocks into one PSUM region. The rT4 drain is
            # the FIRST scalar op of the group so the matmul round trip hides
            # under the entropy activations that follow it.
            psT = psum.tile([128, G * T], F32, tag="psT")
            for k in range(G):
                j = g * G + k
                col = slice(j * T + T - 1, j * T + T)
                nc.gpsimd.tensor_tensor(
                    out=r_s[:, col], in0=r_s[:, col], in1=gr0[:, j : j + 1],
                    op=ALU.add,
                )
                nc.tensor.transpose(
                    psT[:, k * T : (k + 1) * T], r_s[:, j * T : (j + 1) * T],
                    iden_s,
                )
            rT4 = work.tile([128, G * T], F32, tag="rT4", bufs=1)
            nc.scalar.activation(out=rT4, in_=psT, func=ACTF.Copy)

            # R[b, t] = sum_s r'T[s, b] * Lgam[s, t] per block, into one bank
            psR = psum.tile([128, G * T], F32, tag="psR")
            for k in range(G):
                nc.tensor.matmul(
                    psR[:, k * T : (k + 1) * T], lhsT=rT4[:, k * T : (k + 1) * T],
                    rhs=lgam_s, start=True, stop=True,
                )

            # entropy: nbe[b,k] = -beta * sum_{t,a} ent  (per block)
            nbe4 = work.tile([128, G], F32, tag="nbe")
            for k in range(G):
                bs = slice(k * T * A, (k + 1) * T * A)
                nc.scalar.activation(
                    out=enscr, in_=en_t[:, bs], func=ACTF.Copy,
                    bias=0.0, scale=-BETA, accum_out=nbe4[:, k : k + 1],
                )

            # drain R to SBUF on the ACT engine: DVE reads PSUM at ~9 ns/elem
            # (12x slower than SBUF) while ACT reads it at full rate, so the
            # sub below must not touch PSUM from the vector engine
            psRs = work.tile([128, G * T], F32, tag="psRs", bufs=1)
            nc.scalar.activation(out=psRs, in_=psR, func=ACTF.Copy)

            # lp[b, t] = sum_a log_probs; gae telescopes to adv = R - v.
            # Mid-stream the grouped reduce comes first on vector (it releases
            # the lp slot for the SP prefetch). In the tail group the 1.1us
            # sub runs FIRST (its inputs are ready long before the tail data
            # lands) and each block's reduce is emitted right before its STT,
            # so after the last HBM byte only one reduce+STT+combine remain.
            lpr = work.tile([128, G * T], F32, tag="lpr")
            adv4 = work.tile([128, G * T], F32, tag="adv4")
            if tail:
                nc.vector.tensor_sub(adv4, psRs, v_t)
            else:
                nc.vector.reduce_sum(out=lpr, in_=lp_t, axis=AX.X)
                nc.vector.tensor_sub(adv4, psRs, v_t)

            # critic = 0.5 * sum_t adv^2 ; actor = -sum_t lp*adv - beta*sum ent
            acc4 = work.tile([128, G], F32, tag="acc")
            for k in range(G):
                j = g * G + k
                ks = slice(k * T, (k + 1) * T)
                if tail:
                    if k < G - 1:
                        nc.vector.reduce_sum(
                            out=lpr[:, ks], in_=lp_t[:, ks, :], axis=AX.X,
                        )
                    else:
                        for h in range(2):
                            hs = slice(k * T + h * T // 2,
                                       k * T + (h + 1) * T // 2)
                            nc.vector.reduce_sum(
                                out=lpr[:, hs], in_=lp_t[:, hs, :], axis=AX.X,
                            )
                # critic square: mid-stream on vector (keeps the ACT stream
                # short — it releases the en slot); in the tail group on the
                # ACT engine (inputs ready early there) so vector's post-
                # stream chain stays minimal
                if tail:
                    nc.scalar.activation(
                        out=sqscr, in_=adv4[:, ks], func=ACTF.Square,
                        bias=0.0, scale=float(np.sqrt(0.5)),
                        accum_out=stage[:, 2 * j + 1 : 2 * j + 2],
                    )
                else:
                    nc.vector.scalar_tensor_tensor(
                        out=sqscr, in0=adv4[:, ks], scalar=0.5, in1=adv4[:, ks],
                        op0=ALU.mult, op1=ALU.mult,
                        accum_out=stage[:, 2 * j + 1 : 2 * j + 2],
                    )
                nc.vector.scalar_tensor_tensor(
                    out=prod, in0=adv4[:, ks], scalar=-1.0, in1=lpr[:, ks],
                    op0=ALU.mult, op1=ALU.mult, accum_out=acc4[:, k : k + 1],
                )
                # actor combine: mid-stream on vector right behind its own STT
                # (no cross-engine hop); in the tail on gpsimd so it overlaps
                # vector's remaining per-block STTs
                (nc.gpsimd if tail else nc.vector).tensor_tensor(
                    out=stage[:, 2 * j : 2 * j + 1], in0=acc4[:, k : k + 1],
                    in1=nbe4[:, k : k + 1], op=ALU.add,
                )
                if tail and k == G - 2:
                    # blocks 0..62 are final and the input stream is within
                    # one block of done: store them now so the last store is
                    # 8 B/partition
                    nc.scalar.dma_start(out=out_view[:, : 2 * KB - 2],
                                        in_=stage[:, : 2 * KB - 2])

        # last block's two columns at the end on the SP ring (empty by now, so
        # its descriptor generation overlaps store A's data on the ACT ring)
        nc.sync.dma_start(out=out_view[:, 2 * KB - 2 :], in_=stage[:, 2 * KB - 2 :])

    nc.compile()
    return nc


_NC = None


def _get_nc():
    global _NC
    if _NC is None:
        _NC = build_nc(BC)
    return _NC


def _make_in_maps(inputs: dict) -> list[dict]:
    v = np.ascontiguousarray(np.asarray(inputs["values"], dtype=np.float32))
    lv = np.ascontiguousarray(np.asarray(inputs["last_value"], dtype=np.float32))
    r = np.ascontiguousarray(np.asarray(inputs["rewards"], dtype=np.float32))
    lp = np.ascontiguousarray(np.asarray(inputs["log_probs"], dtype=np.float32))
    en = np.ascontiguousarray(np.asarray(inputs["entropies"], dtype=np.float32))
    tm = np.ascontiguousarray(np.asarray(inputs["terminal_mask"]).astype(np.uint8))
    maps = []
    for c in range(N_CORES):
        sl = slice(c * BC, (c + 1) * BC)
        maps.append(
            {
                "values": v[sl],
                "last_value": lv[sl],
                "rewards": r[sl],
                "log_probs": lp[sl],
                "entropies": en[sl],
                "terminal_mask": tm[sl],
            }
        )
    return maps


def _run(inputs: dict, trace: bool = False):
    nc = _get_nc()
    res = run_bass_kernel_spmd(
        nc,
        _make_in_maps(inputs),
        core_ids=list(range(N_CORES)),
        trace=trace,
    )
    out = np.concatenate([res.results[c]["out"] for c in range(N_CORES)], axis=0)
    return out, res


def kernel(**inputs) -> np.ndarray:
    out, _ = _run(inputs, trace=False)
    return out


# revision 37
# speedup vs baseline: 1.2293x; 1.2293x over previous
"""A3C loss kernel for Trainium2 (8 NeuronCores, data-parallel over batch).

The reference is a reverse scan over T=128 timesteps per trajectory:
    R_t   = sum_{s>=t} g^(s-t) r_s + g^(T-t) R0
    gae_t = sum_{s>=t} g^(s-t) delta_s,  delta_s = r_s + g*v_{s+1} - v_s (v_T = R0)
    critic = 0.5 * sum_t (R_t - v_t)^2
    actor  = -sum_t lp_t * gae_t - beta * sum_{t,a} ent
Both suffix scans are matmuls with a [T,T] discount matrix (gae telescopes to
adv = R - v), so the loss is pure streaming: 72 MB/core of HBM reads feeding a
handful of reductions and two small matmuls per 128-row block.

This version is tuned for DMA throughput (the kernel is hard memory-bound;
measured ~424 GB/s/core sustained): blocks are loaded in groups of 8 so each
dma_start moves 32 KB-contiguous chunks per partition, every stream load is
issued from the SP sequencer (which executes nothing but dma_starts, so DMA
issue never serializes behind compute), rewards stay resident in SBUF for the
whole kernel, and the final group is loaded per-block so the dependency chain
after the last HBM byte is short.
"""

import numpy as np
from contextlib import ExitStack

import concourse.bacc as bacc
import concourse.bass as bass
import concourse.tile as tile
from concourse import mybir
from concourse.bass_utils import run_bass_kernel_spmd

GAMMA = 0.99
BETA = 0.01
B, T, A = 65536, 128, 8
N_CORES = 8
BC = B // N_CORES

F32 = mybir.dt.float32
ALU = mybir.AluOpType
ACTF = mybir.ActivationFunctionType
AX = mybir.AxisListType

G = 8            # blocks per DMA group
KB = BC // 128   # 64 blocks of 128 rows
NG = KB // G     # 8 groups


def _discount_matrix() -> np.ndarray:
    # L[s, t] = gamma^(s-t) for s >= t else 0
    s = np.arange(T, dtype=np.float64)[:, None]
    t = np.arange(T, dtype=np.float64)[None, :]
    m = np.where(s >= t, GAMMA ** np.maximum(s - t, 0.0), 0.0)
    return m.astype(np.float32)


def build_nc(bc: int = BC):
    assert bc == 128 * KB

    nc = bacc.Bacc("TRN2", target_bir_lowering=False, debug=False)

    v_d = nc.dram_tensor("values", [bc, T], F32, kind="ExternalInput")
    lv_d = nc.dram_tensor("last_value", [bc], F32, kind="ExternalInput")
    r_d = nc.dram_tensor("rewards", [bc, T], F32, kind="ExternalInput")
    lp_d = nc.dram_tensor("log_probs", [bc, T, A], F32, kind="ExternalInput")
    en_d = nc.dram_tensor("entropies", [bc, T, A], F32, kind="ExternalInput")
    tm_d = nc.dram_tensor("terminal_mask", [bc], mybir.dt.uint8, kind="ExternalInput")
    out_d = nc.dram_tensor("out", [bc, 2], F32, kind="ExternalOutput")

    lgam_d = nc.inline_tensor(_discount_matrix(), "lgam")
    iden_d = nc.inline_tensor(np.eye(128, dtype=np.float32), "iden")

    # row b = p*KB + g*G + k  (partition p, group g, block j = g*G+k)
    lp_view = lp_d.rearrange("(p g k) t a -> g p (k t a)", g=NG, k=G)
    en_view = en_d.rearrange("(p g k) t a -> g p (k t a)", g=NG, k=G)
    v_view = v_d.rearrange("(p g k) t -> g p (k t)", g=NG, k=G)
    r_view = r_d.rearrange("(p j) t -> p (j t)", j=KB)
    lv_view = lv_d.rearrange("(p j) -> p j", j=KB)
    tm_view = tm_d.rearrange("(p j) -> p j", j=KB)
    out_view = out_d.rearrange("(p j) c -> p (j c)", j=KB)

    with tile.TileContext(nc) as tc, ExitStack() as ctx:
        singles = ctx.enter_context(tc.tile_pool(name="singles", bufs=1))
        streams = ctx.enter_context(tc.tile_pool(name="streams", bufs=2))
        work = ctx.enter_context(tc.tile_pool(name="work", bufs=2))
        psum = ctx.enter_context(tc.tile_pool(name="psum", bufs=2, space="PSUM"))

        # write-only scratch blocks: engines visit them serially, so one
        # block-sized dummy per op kind is enough (saves SBUF for the 4 MB
        # double-buffered stream tiles)
        enscr = singles.tile([128, T * A], F32)
        sqscr = singles.tile([128, T], F32)
        prod = singles.tile([128, T], F32)

        # rewards stay resident: one 4 MB load (32 KB/partition), issued first
        # ON THE SP RING so it drains at full rate before the streams (on the
        # ACT ring it would share engine bandwidth with the SP stream and
        # finish ~10us later, stalling the group-0 chain). ALL stream loads go
        # through the SP sequencer: it executes nothing but dma_starts, so it
        # runs ahead of compute and keeps the SDMA engines fed.
        r_s = singles.tile([128, KB * T], F32)
        nc.sync.dma_start(out=r_s, in_=r_view)

        # small one-time loads go through SWDGE (gpsimd) so they stay out of
        # both HWDGE FIFOs that carry the big streams
        lgam_s = singles.tile([128, 128], F32)
        nc.gpsimd.dma_start(out=lgam_s, in_=lgam_d[:])
        iden_s = singles.tile([128, 128], F32)
        nc.gpsimd.dma_start(out=iden_s, in_=iden_d[:])
        lv_s = singles.tile([128, KB], F32)
        nc.gpsimd.dma_start(out=lv_s, in_=lv_view)
        tm_s = singles.tile([128, KB], mybir.dt.uint8)
        nc.gpsimd.dma_start(out=tm_s, in_=tm_view)

        # gr0 = gamma * last_value * (1 - mask)
        tmf = singles.tile([128, KB], F32)
        nc.gpsimd.tensor_copy(out=tmf, in_=tm_s)
        lvm = singles.tile([128, KB], F32)
        nc.gpsimd.tensor_mul(lvm, lv_s, tmf)
        gr0 = singles.tile([128, KB], F32)
        nc.gpsimd.tensor_sub(gr0, lv_s, lvm)
        nc.gpsimd.tensor_scalar_mul(gr0, gr0, GAMMA)

        stage = singles.tile([128, 2 * KB], F32)

        for g in range(NG):
            tail = g == NG - 1
            gc = slice(g * G * T, (g + 1) * G * T)  # this group's v/r columns

            # loads (all on SP). The last group loads per-block so the
            # post-stream dependency chain is short. v gets an extra buffer:
            # its slot is released by the sub at the END of the per-group
            # dependency chain, so bufs=2 couples the prefetch to the chain.
            v_t = streams.tile([128, G * T], F32, tag="v", bufs=3)
            nc.sync.dma_start(out=v_t, in_=v_view[g])
            lp_t = streams.tile([128, G * T, A], F32, tag="lp")
            en_t = streams.tile([128, G * T * A], F32, tag="en")
            if tail:
                for k in range(G):
                    bs = slice(k * T * A, (k + 1) * T * A)
                    nc.sync.dma_start(out=en_t[:, bs], in_=en_view[g][:, bs])
                    if k < G - 1:
                        nc.sync.dma_start(out=lp_t[:, k * T : (k + 1) * T, :],
                                          in_=lp_view[g][:, bs])
                    else:
                        # halve the very last load so the post-stream reduce
                        # chain starts on the first half early
                        for h in range(2):
                            hh = slice(k * T * A + h * T * A // 2,
                                       k * T * A + (h + 1) * T * A // 2)
                            nc.sync.dma_start(
                                out=lp_t[:, k * T + h * T // 2 : k * T + (h + 1) * T // 2, :],
                                in_=lp_view[g][:, hh])
            else:
                nc.sync.dma_start(out=en_t, in_=en_view[g])
                nc.sync.dma_start(out=lp_t, in_=lp_view[g])

            # fold gamma*R0 into the last timestep of r, then time-major
            # transpose of all blocks into one PSUM region. The rT4 drain is
            # the FIRST scalar op of the group so the matmul round trip hides
            # under the entropy activations that follow it.
            psT = psum.tile([128, G * T], F32, tag="psT")
            for k in range(G):
                j = g * G + k
                col = slice(j * T + T - 1, j * T + T)
                nc.gpsimd.tensor_tensor(
                    out=r_s[:, col], in0=r_s[:, col], in1=gr0[:, j : j + 1],
                    op=ALU.add,
                )
                nc.tensor.transpose(
                    psT[:, k * T : (k + 1) * T], r_s[:, j * T : (j + 1) * T],
                    iden_s,
                )
            rT4 = work.tile([128, G * T], F32, tag="rT4", bufs=1)
            nc.scalar.activation(out=rT4, in_=psT, func=ACTF.Copy)

            # R[b, t] = sum_s r'T[s, b] * Lgam[s, t] per block, into one bank
            psR = psum.tile([128, G * T], F32, tag="psR")
            for k in range(G):
                nc.tensor.matmul(
                    psR[:, k * T : (k + 1) * T], lhsT=rT4[:, k * T : (k + 1) * T],
                    rhs=lgam_s, start=True, stop=True,
                )

            # entropy: nbe[b,k] = -beta * sum_{t,a} ent  (per block)
            nbe4 = work.tile([128, G], F32, tag="nbe", bufs=3)
            for k in range(G):
                bs = slice(k * T * A, (k + 1) * T * A)
                nc.scalar.activation(
                    out=enscr, in_=en_t[:, bs], func=ACTF.Copy,
                    bias=0.0, scale=-BETA, accum_out=nbe4[:, k : k + 1],
                )

            # drain R to SBUF on the ACT engine: DVE reads PSUM at ~9 ns/elem
            # (12x slower than SBUF) while ACT reads it at full rate, so the
            # sub below must not touch PSUM from the vector engine
            psRs = work.tile([128, G * T], F32, tag="psRs", bufs=2)
            nc.scalar.activation(out=psRs, in_=psR, func=ACTF.Copy)

            # lp[b, t] = sum_a log_probs; gae telescopes to adv = R - v.
            # Mid-stream the grouped reduce comes first on vector (it releases
            # the lp slot for the SP prefetch). In the tail group the 1.1us
            # sub runs FIRST (its inputs are ready long before the tail data
            # lands) and each block's reduce is emitted right before its STT,
            # so after the last HBM byte only one reduce+STT+combine remain.
            lpr = work.tile([128, G * T], F32, tag="lpr")
            adv4 = work.tile([128, G * T], F32, tag="adv4")
            if tail:
                nc.vector.tensor_sub(adv4, psRs, v_t)
            else:
                nc.vector.reduce_sum(out=lpr, in_=lp_t, axis=AX.X)
                nc.vector.tensor_sub(adv4, psRs, v_t)

            # critic = 0.5 * sum_t adv^2 ; actor = -sum_t lp*adv - beta*sum ent
            acc4 = work.tile([128, G], F32, tag="acc", bufs=3)
            for k in range(G):
                j = g * G + k
                ks = slice(k * T, (k + 1) * T)
                if tail:
                    if k < G - 1:
                        nc.vector.reduce_sum(
                            out=lpr[:, ks], in_=lp_t[:, ks, :], axis=AX.X,
                        )
                    else:
                        for h in range(2):
                            hs = slice(k * T + h * T // 2,
                                       k * T + (h + 1) * T // 2)
                            nc.vector.reduce_sum(
                                out=lpr[:, hs], in_=lp_t[:, hs, :], axis=AX.X,
                            )
                # critic square: mid-stream on vector (keeps the ACT stream
                # short — it releases the en slot); in the tail group on the
                # ACT engine (inputs ready early there) so vector's post-
                # stream chain stays minimal
                if tail:
                    nc.scalar.activation(
                        out=sqscr, in_=adv4[:, ks], func=ACTF.Square,
                        bias=0.0, scale=float(np.sqrt(0.5)),
                        accum_out=stage[:, 2 * j + 1 : 2 * j + 2],
                    )
                else:
                    nc.vector.scalar_tensor_tensor(
                        out=sqscr, in0=adv4[:, ks], scalar=0.5, in1=adv4[:, ks],
                        op0=ALU.mult, op1=ALU.mult,
                        accum_out=stage[:, 2 * j + 1 : 2 * j + 2],
                    )
                nc.vector.scalar_tensor_tensor(
                    out=prod, in0=adv4[:, ks], scalar=-1.0, in1=lpr[:, ks],
                    op0=ALU.mult, op1=ALU.mult, accum_out=acc4[:, k : k + 1],
                )
                # actor combine: mid-stream on vector right behind its own STT
                # (no cross-engine hop); in the tail on gpsimd so it overlaps
                # vector's remaining per-block STTs
                (nc.gpsimd if tail else nc.vector).tensor_tensor(
                    out=stage[:, 2 * j : 2 * j + 1], in0=acc4[:, k : k + 1],
                    in1=nbe4[:, k : k + 1], op=ALU.add,
                )
                if tail and k == G - 2:
                    # blocks 0..62 are final and the input stream is within
                    # one block of done: store them now so the last store is
                    # 8 B/partition
                    nc.scalar.dma_start(out=out_view[:, : 2 * KB - 2],
                                        in_=stage[:, : 2 * KB - 2])

        # last block's two columns at the end on the SP ring (empty by now, so
        # its descriptor generation overlaps store A's data on the ACT ring)
        nc.sync.dma_start(out=out_view[:, 2 * KB - 2 :], in_=stage[:, 2 * KB - 2 :])

    nc.compile()
    return nc


_NC = None


def _get_nc():
    global _NC
    if _NC is None:
        _NC = build_nc(BC)
    return _NC


def _make_in_maps(inputs: dict) -> list[dict]:
    v = np.ascontiguousarray(np.asarray(inputs["values"], dtype=np.float32))
    lv = np.ascontiguousarray(np.asarray(inputs["last_value"], dtype=np.float32))
    r = np.ascontiguousarray(np.asarray(inputs["rewards"], dtype=np.float32))
    lp = np.ascontiguousarray(np.asarray(inputs["log_probs"], dtype=np.float32))
    en = np.ascontiguousarray(np.asarray(inputs["entropies"], dtype=np.float32))
    tm = np.ascontiguousarray(np.asarray(inputs["terminal_mask"]).astype(np.uint8))
    maps = []
    for c in range(N_CORES):
        sl = slice(c * BC, (c + 1) * BC)
        maps.append(
            {
                "values": v[sl],
                "last_value": lv[sl],
                "rewards": r[sl],
                "log_probs": lp[sl],
                "entropies": en[sl],
                "terminal_mask": tm[sl],
            }
        )
    return maps


def _run(inputs: dict, trace: bool = False):
    nc = _get_nc()
    res = run_bass_kernel_spmd(
        nc,
        _make_in_maps(inputs),
        core_ids=list(range(N_CORES)),
        trace=trace,
    )
    out = np.concatenate([res.results[c]["out"] for c in range(N_CORES)], axis=0)
    return out, res


def kernel(**inputs) -> np.ndarray:
    out, _ = _run(inputs, trace=False)
    return out
